# revision 3
# baseline (speedup 1.0000x reference)
"""Chamfer image loss kernel for Trainium2 (8 NeuronCores, SPMD).

Problem: M=N=16384 2-D points. loss = mean_m min_n ||x_m - y_n||^2
                                      + mean_n min_m ||x_m - y_n||^2
where x = perspective-projected points and y = mask samples.

Since the reference gathers the argmin neighbor and recomputes the exact
squared distance, the loss equals the row/col minima of the expanded-form
distance matrix up to fp32 rounding (validated: ~1e-7 rel).  So no
argmin/gather is needed on device - only min-reductions.

Device strategy (sharding: M axis across 8 cores, M_loc = 2048):
  d2[m,n] = sum_k A[k,m] * B[k,n]   with K=4 augmented vectors
      A = [x0, x1, ||x||^2, 1],  B = [-2 y0, -2 y1, 1, ||y||^2]
  Each fp32 component is split into 3 bf16 terms (h/m/l); the 6 product
  groups hh, hm, mh, hl, lh, mm are stacked into K=24 bf16 rows, so the
  PE runs at full bf16 rate with ~2^-27 relative error (better than a
  plain fp32 matmul).  PSUM accumulates fp32.
  - p2gt: per 128-row x tile, stream all y in [128, 2048] PSUM chunks,
    DVE min-reduce each chunk, combine -> 2048 row minima per core.
  - gt2p: transposed matmuls (y stationary, x moving) -> per-core column
    minima over its 2048 x's for all 16384 y's.
Host glue: input projection/augmentation/sharding, 8-way min combine of
column minima, and the two means.
"""

import sys

for _p in ("/opt/trn_rl_repo",):
    if _p not in sys.path:
        sys.path.insert(0, _p)

import numpy as np
import ml_dtypes

import concourse.bass as bass
import concourse.mybir as mybir
from concourse.tile import TileContext
from concourse.vector_clock import ScopedClock
from concourse.bass_utils import run_bass_kernel_spmd

bf16 = ml_dtypes.bfloat16

IMG_W, IMG_H = 640, 480
FX = np.float32(600.0 / IMG_W)
FY = np.float32(600.0 / IMG_H)

M = 16384
N = 16384
N_CORES = 8
M_LOC = M // N_CORES  # 2048
K = 24  # 6 bf16 product groups x 4 augmented components
CHUNK = 2048  # PSUM chunk free size (4 banks)
MM_N = 512  # matmul moving free size (1 PSUM bank)


class SplitDrainTileContext(TileContext):
    """This walrus build accepts a single sem wait per instruction.  Tile
    attaches one wait per required proc to the consuming instruction, so
    legalize: keep one wait on the instruction and move the rest onto
    preceding same-engine NOPs (raw-bass style standalone waits)."""

    def _add_instruction(self, inst):
        si = inst.sync_info
        if si is not None and si.on_wait and len(si.on_wait) > 1:
            waits = list(si.on_wait)
            inst.sync_info = mybir.SyncInfo(
                on_wait=waits[-1:], on_update=list(si.on_update or [])
            )
            for w in waits[:-1]:
                nop = mybir.InstNoOp(
                    name=self.nc.get_next_instruction_name(),
                    engine=inst.engine,
                    sync_info=mybir.SyncInfo(on_wait=[w], on_update=[]),
                    bass_nofuse=True,
                )
                super()._add_instruction(nop)
        super()._add_instruction(inst)

    def _drain_and_barrier(self, tick_clock, wait_clock):
        nc = self.nc
        drain_inst = nc.sync.drain()
        wait_clock.add_sem_waits(
            drain_inst.ins, ScopedClock({None: tick_clock.global_clock})
        )
        si = drain_inst.ins.sync_info
        if si is not None and si.on_wait and len(si.on_wait) > 1:
            waits = list(si.on_wait)
            si.on_wait = waits[:1]
            for w in waits[1:]:
                extra = nc.sync.drain()
                extra.ins.sync_info = mybir.SyncInfo(on_wait=[w], on_update=[])
        nc.all_engine_barrier()
        assert self.sems is not None
        popped = nc._tile_sem_poison_stack.pop()
        assert popped is self._sem_poison
        nc.clear_and_free_semaphores(list(self.sems.allocated().values()))
        nc.all_engine_barrier()


def _build_program():
    nc = bass.Bass()
    xa = nc.dram_tensor("xa", [K, M_LOC], mybir.dt.bfloat16, kind="ExternalInput")
    ya = nc.dram_tensor("ya", [K, N], mybir.dt.bfloat16, kind="ExternalInput")
    yb = nc.dram_tensor("yb", [K, N], mybir.dt.bfloat16, kind="ExternalInput")
    xb = nc.dram_tensor("xb", [K, M_LOC], mybir.dt.bfloat16, kind="ExternalInput")
    rowmin = nc.dram_tensor("rowmin", [M_LOC], mybir.dt.float32, kind="ExternalOutput")
    colmin = nc.dram_tensor("colmin", [N], mybir.dt.float32, kind="ExternalOutput")

    n_xt = M_LOC // 128  # 16 x tiles
    n_yt = N // 128  # 128 y tiles
    n_chunks = N // CHUNK  # 8 y chunks per x tile

    with SplitDrainTileContext(nc) as tc:
        with (
            tc.tile_pool(name="inp", bufs=1) as inp,
            tc.tile_pool(name="acc", bufs=1) as acc,
            tc.tile_pool(name="ps", bufs=2, space="PSUM") as ps,
        ):
            xa_t = inp.tile([K, M_LOC], mybir.dt.bfloat16)
            ya_t = inp.tile([K, N], mybir.dt.bfloat16)
            yb_t = inp.tile([K, N], mybir.dt.bfloat16)
            xb_t = inp.tile([K, M_LOC], mybir.dt.bfloat16)
            nc.sync.dma_start(out=xa_t, in_=xa[:, :])
            nc.sync.dma_start(out=ya_t, in_=ya[:, :])
            nc.sync.dma_start(out=yb_t, in_=yb[:, :])
            nc.sync.dma_start(out=xb_t, in_=xb[:, :])

            # p2gt: rowmin over all y for each of this core's 2048 x's
            rp = acc.tile([128, n_xt * n_chunks], mybir.dt.float32)  # [128, 128]
            for xt in range(n_xt):
                lhs = xa_t[:, xt * 128 : (xt + 1) * 128]
                for c in range(n_chunks):
                    d2 = ps.tile([128, CHUNK], mybir.dt.float32, tag="d2")
                    for j in range(CHUNK // MM_N):
                        off = c * CHUNK + j * MM_N
                        nc.tensor.matmul(
                            out=d2[:, j * MM_N : (j + 1) * MM_N],
                            lhsT=lhs,
                            rhs=ya_t[:, off : off + MM_N],
                            start=True,
                            stop=True,
                        )
                    col = xt * n_chunks + c
                    nc.vector.tensor_reduce(
                        out=rp[:, col : col + 1],
                        in_=d2[:, :],
                        axis=mybir.AxisListType.X,
                        op=mybir.AluOpType.min,
                    )
            rm = acc.tile([128, n_xt], mybir.dt.float32)
            nc.vector.tensor_reduce(
                out=rm,
                in_=rp.rearrange("p (t c) -> p t c", c=n_chunks),
                axis=mybir.AxisListType.X,
                op=mybir.AluOpType.min,
            )
            # rowmin index m = xt*128 + p
            nc.sync.dma_start(
                out=rowmin.rearrange("(t p) -> p t", p=128), in_=rm[:, :]
            )

            # gt2p: colmin over this core's x's for each of the 16384 y's
            cm = acc.tile([128, n_yt], mybir.dt.float32)  # [128, 128]
            for yt in range(n_yt):
                lhs = yb_t[:, yt * 128 : (yt + 1) * 128]
                d2g = ps.tile([128, CHUNK], mybir.dt.float32, tag="d2")
                for j in range(M_LOC // MM_N):
                    nc.tensor.matmul(
                        out=d2g[:, j * MM_N : (j + 1) * MM_N],
                        lhsT=lhs,
                        rhs=xb_t[:, j * MM_N : (j + 1) * MM_N],
                        start=True,
                        stop=True,
                    )
                nc.vector.tensor_reduce(
                    out=cm[:, yt : yt + 1],
                    in_=d2g[:, :],
                    axis=mybir.AxisListType.X,
                    op=mybir.AluOpType.min,
                )
            # colmin index n = yt*128 + p
            nc.sync.dma_start(
                out=colmin.rearrange("(t p) -> p t", p=128), in_=cm[:, :]
            )
    return nc


_NC_CACHE = None


def _get_program():
    global _NC_CACHE
    if _NC_CACHE is None:
        _NC_CACHE = _build_program()
    return _NC_CACHE


def _split3(a):
    """fp32 -> 3 bf16 terms whose sum reproduces a to ~2^-27 rel."""
    a = a.astype(np.float32)
    h = a.astype(bf16)
    r1 = (a - h.astype(np.float32)).astype(np.float32)
    m = r1.astype(bf16)
    l = (r1 - m.astype(np.float32)).astype(bf16)
    return h, m, l


def _stack_split(stat4, mov4):
    """(4, n) fp32 stationary/moving components -> (24, n) bf16 stacks
    covering product groups hh, hm, mh, hl, lh, mm."""
    sh, sm, sl = _split3(stat4)
    mh, mm_, ml = _split3(mov4)
    stat = np.concatenate([sh, sh, sm, sh, sl, sm], axis=0).astype(bf16)
    mov = np.concatenate([mh, mm_, mh, ml, mh, mm_], axis=0).astype(bf16)
    return stat, mov


def kernel(input, mask_samples, norm_scale, norm_shift):
    x3 = np.asarray(input, dtype=np.float32)
    y = np.asarray(mask_samples, dtype=np.float32)[0]
    sc = np.asarray(norm_scale, dtype=np.float32)
    sh = np.asarray(norm_shift, dtype=np.float32)

    cam = (x3 * sc + sh).astype(np.float32)
    pred = (
        np.stack([cam[:, 0] * FX, cam[:, 1] * FY], axis=-1) / cam[:, 2:3]
    ).astype(np.float32)

    xn = (pred * pred).sum(axis=1, dtype=np.float32).astype(np.float32)
    yn = (y * y).sum(axis=1, dtype=np.float32).astype(np.float32)
    ones_m = np.ones(M, np.float32)
    ones_n = np.ones(N, np.float32)

    # direction 1 (p2gt): stationary x, moving y
    a4 = np.stack([pred[:, 0], pred[:, 1], xn, ones_m], axis=0)  # (4, M)
    b4 = np.stack([-2.0 * y[:, 0], -2.0 * y[:, 1], ones_n, yn], axis=0)  # (4, N)
    xa_full, ya_full = _stack_split(a4, b4)
    # direction 2 (gt2p): stationary y, moving x
    c4 = np.stack([y[:, 0], y[:, 1], yn, ones_n], axis=0)  # (4, N)
    d4 = np.stack([-2.0 * pred[:, 0], -2.0 * pred[:, 1], ones_m, xn], axis=0)
    yb_full, xb_full = _stack_split(c4, d4)

    in_maps = []
    for c in range(N_CORES):
        s = slice(c * M_LOC, (c + 1) * M_LOC)
        in_maps.append(
            {
                "xa": np.ascontiguousarray(xa_full[:, s]),
                "ya": np.ascontiguousarray(ya_full),
                "yb": np.ascontiguousarray(yb_full),
                "xb": np.ascontiguousarray(xb_full[:, s]),
            }
        )

    global _last_in_maps
    _last_in_maps = in_maps
    nc = _get_program()
    res = run_bass_kernel_spmd(nc, in_maps, core_ids=list(range(N_CORES)))

    rowmins = np.concatenate([res.results[c]["rowmin"] for c in range(N_CORES)])
    colmin = np.min(
        np.stack([res.results[c]["colmin"] for c in range(N_CORES)], axis=0), axis=0
    )
    loss = np.float32(
        rowmins.mean(dtype=np.float64) + colmin.mean(dtype=np.float64)
    )
    return np.asarray(loss, dtype=np.float32)


if __name__ == "__main__":
    d = np.load("/root/problem/inputs.npz")
    out = kernel(**{k: d[k] for k in d.files})
    print("loss:", out)


# revision 4
# speedup vs baseline: 6.1148x; 6.1148x over previous
"""Chamfer image loss kernel for Trainium2 (8 NeuronCores, SPMD).

loss = mean_m min_n ||x_m - y_n||^2 + mean_n min_m ||x_m - y_n||^2 with
x = perspective-projected `input` points and y = mask samples
(M = N = 16384).  The reference gathers the argmin neighbor and
recomputes the exact squared distance, so the loss equals the row/col
minima of the expanded-form distance matrix up to fp32 rounding
(validated ~1e-7 rel) - no argmin/gather needed.

Strategy: band-pruned nearest neighbor.
  Host planning (numpy, O((M+N) * sqrt(N))-ish):
   - Sort each database into 32 equal-count rows by coord1, by coord0
     within each row.  Sort queries by (db row, coord0); tile by 128.
   - A coarse probe (512-point db subsample) upper-bounds each query's
     NN distance; per-tile window half-width W comes from the exact ball
     bound sqrt(ub^2 + 2*dist_outside*ub), so each tile's candidate set
     (per-row contiguous runs, gathered dense) provably contains every
     query's NN except for rare boundary cases.
   - Candidates are packed into 512-wide chunks; both directions share
     one flat chunk stream, split evenly across the 8 cores.
  Device (per core): for each chunk, one K=24 bf16 matmul forms the
  expanded-form d2 block (each fp32 component is split into 3 bf16
  terms; the 6 product groups hh,hm,mh,hl,lh,mm make the matmul exact to
  ~2^-27, better than fp32), and the DVE min-reduces 4 chunks per PSUM
  tile into per-chunk partial minima.
  Host epilogue: combine partials per tile, run a conservative 4-sided
  gap check (uncovered region distance bound); the few failures are
  recomputed exactly on host.  Means are order-invariant, so the query
  sort never needs undoing.
"""

import sys

for _p in ("/opt/trn_rl_repo",):
    if _p not in sys.path:
        sys.path.insert(0, _p)

import numpy as np
import ml_dtypes

import concourse.bass as bass
import concourse.mybir as mybir
from concourse.tile import TileContext
from concourse.vector_clock import ScopedClock
from concourse.bass_utils import run_bass_kernel_spmd

bf16 = ml_dtypes.bfloat16

IMG_W, IMG_H = 640, 480
FX = np.float32(600.0 / IMG_W)
FY = np.float32(600.0 / IMG_H)

M = 16384
N = 16384
N_CORES = 8
TILE = 128
K = 24  # 6 bf16 product groups x 4 augmented components
CHUNK = 512  # candidates per chunk (one matmul / PSUM bank)
GROUP = 4  # chunks per PSUM tile / DVE reduce
R_ROWS = 32


class SplitDrainTileContext(TileContext):
    """This walrus build accepts a single sem wait per instruction.  Tile
    attaches one wait per required proc to the consuming instruction, so
    legalize: keep one wait on the instruction and move the rest onto
    preceding same-engine NOPs (raw-bass style standalone waits)."""

    def _add_instruction(self, inst):
        si = inst.sync_info
        if si is not None and si.on_wait and len(si.on_wait) > 1:
            waits = list(si.on_wait)
            inst.sync_info = mybir.SyncInfo(
                on_wait=waits[-1:], on_update=list(si.on_update or [])
            )
            for w in waits[:-1]:
                nop = mybir.InstNoOp(
                    name=self.nc.get_next_instruction_name(),
                    engine=inst.engine,
                    sync_info=mybir.SyncInfo(on_wait=[w], on_update=[]),
                    bass_nofuse=True,
                )
                super()._add_instruction(nop)
        super()._add_instruction(inst)

    def _drain_and_barrier(self, tick_clock, wait_clock):
        nc = self.nc
        drain_inst = nc.sync.drain()
        wait_clock.add_sem_waits(
            drain_inst.ins, ScopedClock({None: tick_clock.global_clock})
        )
        si = drain_inst.ins.sync_info
        if si is not None and si.on_wait and len(si.on_wait) > 1:
            waits = list(si.on_wait)
            si.on_wait = waits[:1]
            for w in waits[1:]:
                extra = nc.sync.drain()
                extra.ins.sync_info = mybir.SyncInfo(on_wait=[w], on_update=[])
        nc.all_engine_barrier()
        assert self.sems is not None
        popped = nc._tile_sem_poison_stack.pop()
        assert popped is self._sem_poison
        nc.clear_and_free_semaphores(list(self.sems.allocated().values()))
        nc.all_engine_barrier()


_PROGRAMS = {}


def _get_program(n_groups):
    """Device program: n_groups x (GROUP matmuls into one PSUM tile + one
    3D-AP min reduce).  Cached per n_groups."""
    if n_groups in _PROGRAMS:
        return _PROGRAMS[n_groups]
    ch = n_groups * GROUP
    nc = bass.Bass()
    qflat = nc.dram_tensor("qflat", [K, ch * TILE], mybir.dt.bfloat16, kind="ExternalInput")
    cflat = nc.dram_tensor("cflat", [K, ch * CHUNK], mybir.dt.bfloat16, kind="ExternalInput")
    pm = nc.dram_tensor("pm", [TILE, ch], mybir.dt.float32, kind="ExternalOutput")

    with SplitDrainTileContext(nc) as tc:
        with (
            tc.tile_pool(name="inp", bufs=1) as inp,
            tc.tile_pool(name="cbuf", bufs=4) as cbuf,
            tc.tile_pool(name="acc", bufs=1) as acc,
            tc.tile_pool(name="ps", bufs=2, space="PSUM") as ps,
        ):
            q_sb = inp.tile([K, ch * TILE], mybir.dt.bfloat16)
            nc.sync.dma_start(out=q_sb, in_=qflat[:, :])
            pm_sb = acc.tile([TILE, ch], mybir.dt.float32)
            for g in range(n_groups):
                c_sb = cbuf.tile([K, GROUP * CHUNK], mybir.dt.bfloat16, tag="cand")
                nc.sync.dma_start(
                    out=c_sb,
                    in_=cflat[:, g * GROUP * CHUNK : (g + 1) * GROUP * CHUNK],
                )
                d2 = ps.tile([TILE, GROUP * CHUNK], mybir.dt.float32, tag="d2")
                for s in range(GROUP):
                    j = g * GROUP + s
                    nc.tensor.matmul(
                        out=d2[:, s * CHUNK : (s + 1) * CHUNK],
                        lhsT=q_sb[:, j * TILE : (j + 1) * TILE],
                        rhs=c_sb[:, s * CHUNK : (s + 1) * CHUNK],
                        start=True,
                        stop=True,
                    )
                nc.vector.tensor_reduce(
                    out=pm_sb[:, g * GROUP : (g + 1) * GROUP],
                    in_=d2.rearrange("p (s c) -> p s c", c=CHUNK),
                    axis=mybir.AxisListType.X,
                    op=mybir.AluOpType.min,
                )
            nc.sync.dma_start(out=pm[:, :], in_=pm_sb)
    _PROGRAMS[n_groups] = nc
    return nc


def _split3(a):
    a = np.asarray(a, np.float32)
    h = a.astype(bf16)
    r1 = (a - h.astype(np.float32)).astype(np.float32)
    m = r1.astype(bf16)
    l = (r1 - m.astype(np.float32)).astype(bf16)
    return h, m, l


def _stack_split(stat4, mov4):
    sh, sm, sl = _split3(stat4)
    mh, mm_, ml = _split3(mov4)
    stat = np.concatenate([sh, sh, sm, sh, sl, sm], axis=0).astype(bf16)
    mov = np.concatenate([mh, mm_, mh, ml, mh, mm_], axis=0).astype(bf16)
    return stat, mov


def _build_db(ds):
    n = len(ds)
    o1 = np.argsort(ds[:, 1], kind="stable")
    s = ds[o1]
    starts = (np.arange(R_ROWS + 1) * n) // R_ROWS
    out = np.empty_like(s)
    for r in range(R_ROWS):
        seg = s[starts[r] : starts[r + 1]]
        out[starts[r] : starts[r + 1]] = seg[np.argsort(seg[:, 0], kind="stable")]
    edges = np.empty(R_ROWS + 1, np.float64)
    edges[0] = -np.inf
    for r in range(1, R_ROWS):
        edges[r] = 0.5 * (float(s[starts[r] - 1, 1]) + float(s[starts[r], 1]))
    edges[R_ROWS] = np.inf
    return out, starts, edges


def _plan_direction(qs_raw, ds_raw):
    """Returns dict with sorted queries, candidate chunk indices per tile,
    and the coverage metadata for the conservative check."""
    db, starts, edges = _build_db(ds_raw)
    d0lo, d0hi = float(db[:, 0].min()), float(db[:, 0].max())
    d1lo, d1hi = float(db[:, 1].min()), float(db[:, 1].max())
    qc = np.stack(
        [np.clip(qs_raw[:, 0], d0lo, d0hi), np.clip(qs_raw[:, 1], d1lo, d1hi)], -1
    ).astype(np.float32)
    # probe: NN-distance upper bound from a coarse subsample
    S = db[::32]
    qn = (qc * qc).sum(1)
    sn = (S * S).sum(1)
    ub2 = np.maximum((qn[:, None] - 2.0 * (qc @ S.T) + sn[None, :]).min(1), 0)
    ub = np.sqrt(ub2.astype(np.float64))
    dist_out = np.sqrt(((qs_raw - qc) ** 2).sum(1).astype(np.float64))
    wq = np.sqrt(ub * ub + 2.0 * dist_out * ub)  # exact NN ball about clamp(q)
    qrow = np.searchsorted(edges[1:-1], qs_raw[:, 1], "right")
    oq = np.lexsort((qc[:, 0], qrow))
    qs = qs_raw[oq]
    qcs = qc[oq]
    wqs = wq[oq]
    n_t = len(qs) // TILE
    tiles = []
    for t in range(n_t):
        sl = slice(t * TILE, (t + 1) * TILE)
        W = float(wqs[sl].max()) * 1.05 + 0.003
        q0lo, q0hi = float(qcs[sl, 0].min()), float(qcs[sl, 0].max())
        q1lo, q1hi = float(qcs[sl, 1].min()), float(qcs[sl, 1].max())
        rlo = int(np.searchsorted(edges[1:-1], q1lo - W, "right"))
        rhi = int(np.searchsorted(edges[1:-1], q1hi + W, "right"))
        runs = []
        for r in range(rlo, rhi + 1):
            a, b = int(starts[r]), int(starts[r + 1])
            l = a + int(np.searchsorted(db[a:b, 0], q0lo - W, "left"))
            h = a + int(np.searchsorted(db[a:b, 0], q0hi + W, "right"))
            runs.append((r, l, h))
        parts = [np.arange(l, h) for (_, l, h) in runs if h > l]
        idx = np.concatenate(parts) if parts else np.zeros(1, np.int64)
        tiles.append({"idx": idx, "W": W, "rlo": rlo, "rhi": rhi, "runs": runs})
    return {
        "db": db,
        "starts": starts,
        "edges": edges,
        "qs": qs,
        "oq": oq,
        "tiles": tiles,
        "ds_raw": ds_raw,
    }


def _check_direction(plan, dmin):
    db, starts, edges = plan["db"], plan["starts"], plan["edges"]
    qs = plan["qs"]
    bad = np.zeros(len(qs), bool)
    for t, tl in enumerate(plan["tiles"]):
        sl = slice(t * TILE, (t + 1) * TILE)
        q = qs[sl]
        dm = dmin[sl]
        rlo, rhi = tl["rlo"], tl["rhi"]
        gaps = np.full((TILE, 4), np.inf, np.float64)
        if np.isfinite(edges[rlo]):
            gaps[:, 0] = q[:, 1] - edges[rlo]
        if np.isfinite(edges[rhi + 1]):
            gaps[:, 1] = edges[rhi + 1] - q[:, 1]
        L, Rv = -np.inf, np.inf
        for (r, l, h) in tl["runs"]:
            a, b = int(starts[r]), int(starts[r + 1])
            if l > a:
                L = max(L, float(db[l - 1, 0]))
            if h < b:
                Rv = min(Rv, float(db[h, 0]))
        if np.isfinite(L):
            gaps[:, 2] = q[:, 0] - L
        if np.isfinite(Rv):
            gaps[:, 3] = Rv - q[:, 0]
        g = np.clip(gaps, 0, None).min(axis=1)
        bad[sl] = dm.astype(np.float64) > g * g
    return bad


_last_in_maps = None


def kernel(input, mask_samples, norm_scale, norm_shift):
    global _last_in_maps
    x3 = np.asarray(input, dtype=np.float32)
    y = np.asarray(mask_samples, dtype=np.float32)[0]
    sc = np.asarray(norm_scale, dtype=np.float32)
    sh = np.asarray(norm_shift, dtype=np.float32)

    cam = (x3 * sc + sh).astype(np.float32)
    pred = (
        np.stack([cam[:, 0] * FX, cam[:, 1] * FY], axis=-1) / cam[:, 2:3]
    ).astype(np.float32)

    plans = [_plan_direction(pred, y), _plan_direction(y, pred)]

    # flat chunk stream over both directions
    chunks = []  # (direction, tile, candidate index array of len CHUNK)
    for di, plan in enumerate(plans):
        for t, tl in enumerate(plan["tiles"]):
            idx = tl["idx"]
            n_ch = max(1, (len(idx) + CHUNK - 1) // CHUNK)
            padded = np.empty(n_ch * CHUNK, np.int64)
            padded[: len(idx)] = idx
            if len(idx) < len(padded):
                padded[len(idx) :] = idx[0]
            for j in range(n_ch):
                chunks.append((di, t, padded[j * CHUNK : (j + 1) * CHUNK]))

    per_core = -(-len(chunks) // (N_CORES * GROUP)) * GROUP  # ceil to GROUP
    n_groups = per_core // GROUP
    total = per_core * N_CORES
    while len(chunks) < total:
        chunks.append(chunks[-1])

    # device input stacks per direction: stationary (query) / moving (cands)
    qstacks, cstacks = [], []
    for di, plan in enumerate(plans):
        qs, db = plan["qs"], plan["db"]
        qn = (qs * qs).sum(1, dtype=np.float32)
        dn = (db * db).sum(1, dtype=np.float32)
        ones_q = np.ones(len(qs), np.float32)
        ones_d = np.ones(len(db), np.float32)
        a4 = np.stack([qs[:, 0], qs[:, 1], qn, ones_q], axis=0)
        b4 = np.stack([-2.0 * db[:, 0], -2.0 * db[:, 1], ones_d, dn], axis=0)
        qa, cb = _stack_split(a4, b4)
        qstacks.append(qa)
        cstacks.append(cb)

    in_maps = []
    for c in range(N_CORES):
        sl = chunks[c * per_core : (c + 1) * per_core]
        qcols = np.concatenate(
            [qstacks[di][:, t * TILE : (t + 1) * TILE] for (di, t, _) in sl], axis=1
        )
        ccols = np.concatenate([cstacks[di][:, ci] for (di, _, ci) in sl], axis=1)
        in_maps.append(
            {
                "qflat": np.ascontiguousarray(qcols),
                "cflat": np.ascontiguousarray(ccols),
            }
        )
    _last_in_maps = in_maps

    nc = _get_program(n_groups)
    res = run_bass_kernel_spmd(nc, in_maps, core_ids=list(range(N_CORES)))

    # combine partial minima per (direction, tile)
    dmins = [np.full(M, np.inf, np.float32), np.full(N, np.inf, np.float32)]
    for j, (di, t, _) in enumerate(chunks[: len(chunks)]):
        c, lj = divmod(j, per_core)
        col = res.results[c]["pm"][:, lj]
        sl = slice(t * TILE, (t + 1) * TILE)
        np.minimum(dmins[di][sl], col, out=dmins[di][sl])

    # conservative coverage check + exact host fixup
    for di, plan in enumerate(plans):
        bad = _check_direction(plan, dmins[di])
        if bad.any():
            qb = plan["qs"][bad]
            ds_raw = plan["ds_raw"]
            dn_all = (ds_raw * ds_raw).sum(1, dtype=np.float32)
            qn_b = (qb * qb).sum(1, dtype=np.float32)
            d2 = (
                qn_b[:, None] - 2.0 * (qb @ ds_raw.T) + dn_all[None, :]
            ).astype(np.float32)
            dmins[di][bad] = d2.min(1)

    loss = np.float32(
        dmins[0].mean(dtype=np.float64) + dmins[1].mean(dtype=np.float64)
    )
    return np.asarray(loss, dtype=np.float32)


if __name__ == "__main__":
    d = np.load("/root/problem/inputs.npz")
    out = kernel(**{k: d[k] for k in d.files})
    print("loss:", out)


# revision 5
# speedup vs baseline: 6.1401x; 1.0041x over previous
"""Chamfer image loss kernel for Trainium2 (8 NeuronCores, SPMD).

loss = mean_m min_n ||x_m - y_n||^2 + mean_n min_m ||x_m - y_n||^2 with
x = perspective-projected `input` points and y = mask samples
(M = N = 16384).  The reference gathers the argmin neighbor and
recomputes the exact squared distance, so the loss equals the row/col
minima of the expanded-form distance matrix up to fp32 rounding
(validated ~1e-7 rel) - no argmin/gather needed.

Strategy: band-pruned nearest neighbor.
  Host planning (numpy, O((M+N) * sqrt(N))-ish):
   - Sort each database into 32 equal-count rows by coord1, by coord0
     within each row.  Sort queries by (db row, coord0); tile by 128.
   - A coarse probe (512-point db subsample) upper-bounds each query's
     NN distance; per-tile window half-width W comes from the exact ball
     bound sqrt(ub^2 + 2*dist_outside*ub), so each tile's candidate set
     (per-row contiguous runs, gathered dense) provably contains every
     query's NN except for rare boundary cases.
   - Candidates are packed into 512-wide chunks; both directions share
     one flat chunk stream, split evenly across the 8 cores.
  Device (per core): for each chunk, one K=24 bf16 matmul forms the
  expanded-form d2 block (each fp32 component is split into 3 bf16
  terms; the product groups hh,hm,mh make the matmul exact to
  ~2^-18), and the DVE min-reduces 4 chunks per PSUM
  tile into per-chunk partial minima.
  Host epilogue: combine partials per tile, run a conservative 4-sided
  gap check (uncovered region distance bound); the few failures are
  recomputed exactly on host.  Means are order-invariant, so the query
  sort never needs undoing.
"""

import sys

for _p in ("/opt/trn_rl_repo",):
    if _p not in sys.path:
        sys.path.insert(0, _p)

import numpy as np
import ml_dtypes

import concourse.bass as bass
import concourse.mybir as mybir
from concourse.tile import TileContext
from concourse.vector_clock import ScopedClock
from concourse.bass_utils import run_bass_kernel_spmd

bf16 = ml_dtypes.bfloat16

IMG_W, IMG_H = 640, 480
FX = np.float32(600.0 / IMG_W)
FY = np.float32(600.0 / IMG_H)

M = 16384
N = 16384
N_CORES = 8
TILE = 128
K = 12  # 3 bf16 product groups (hh,hm,mh) x 4 augmented components
CHUNK = 512  # candidates per chunk (one matmul / PSUM bank)
GROUP = 4  # chunks per PSUM tile / DVE reduce
R_ROWS = 32


class SplitDrainTileContext(TileContext):
    """This walrus build accepts a single sem wait per instruction.  Tile
    attaches one wait per required proc to the consuming instruction, so
    legalize: keep one wait on the instruction and move the rest onto
    preceding same-engine NOPs (raw-bass style standalone waits)."""

    def _add_instruction(self, inst):
        si = inst.sync_info
        if si is not None and si.on_wait and len(si.on_wait) > 1:
            waits = list(si.on_wait)
            inst.sync_info = mybir.SyncInfo(
                on_wait=waits[-1:], on_update=list(si.on_update or [])
            )
            for w in waits[:-1]:
                nop = mybir.InstNoOp(
                    name=self.nc.get_next_instruction_name(),
                    engine=inst.engine,
                    sync_info=mybir.SyncInfo(on_wait=[w], on_update=[]),
                    bass_nofuse=True,
                )
                super()._add_instruction(nop)
        super()._add_instruction(inst)

    def _drain_and_barrier(self, tick_clock, wait_clock):
        nc = self.nc
        drain_inst = nc.sync.drain()
        wait_clock.add_sem_waits(
            drain_inst.ins, ScopedClock({None: tick_clock.global_clock})
        )
        si = drain_inst.ins.sync_info
        if si is not None and si.on_wait and len(si.on_wait) > 1:
            waits = list(si.on_wait)
            si.on_wait = waits[:1]
            for w in waits[1:]:
                extra = nc.sync.drain()
                extra.ins.sync_info = mybir.SyncInfo(on_wait=[w], on_update=[])
        nc.all_engine_barrier()
        assert self.sems is not None
        popped = nc._tile_sem_poison_stack.pop()
        assert popped is self._sem_poison
        nc.clear_and_free_semaphores(list(self.sems.allocated().values()))
        nc.all_engine_barrier()


_PROGRAMS = {}


def _get_program(n_groups):
    """Device program: n_groups x (GROUP matmuls into one PSUM tile + one
    3D-AP min reduce).  Cached per n_groups."""
    if n_groups in _PROGRAMS:
        return _PROGRAMS[n_groups]
    ch = n_groups * GROUP
    nc = bass.Bass()
    qflat = nc.dram_tensor("qflat", [K, ch * TILE], mybir.dt.bfloat16, kind="ExternalInput")
    cflat = nc.dram_tensor("cflat", [K, ch * CHUNK], mybir.dt.bfloat16, kind="ExternalInput")
    pm = nc.dram_tensor("pm", [TILE, ch], mybir.dt.float32, kind="ExternalOutput")

    with SplitDrainTileContext(nc) as tc:
        with (
            tc.tile_pool(name="inp", bufs=1) as inp,
            tc.tile_pool(name="cbuf", bufs=4) as cbuf,
            tc.tile_pool(name="acc", bufs=1) as acc,
            tc.tile_pool(name="ps", bufs=2, space="PSUM") as ps,
        ):
            q_sb = inp.tile([K, ch * TILE], mybir.dt.bfloat16)
            nc.sync.dma_start(out=q_sb, in_=qflat[:, :])
            pm_sb = acc.tile([TILE, ch], mybir.dt.float32)
            for g in range(n_groups):
                c_sb = cbuf.tile([K, GROUP * CHUNK], mybir.dt.bfloat16, tag="cand")
                nc.sync.dma_start(
                    out=c_sb,
                    in_=cflat[:, g * GROUP * CHUNK : (g + 1) * GROUP * CHUNK],
                )
                d2 = ps.tile([TILE, GROUP * CHUNK], mybir.dt.float32, tag="d2")
                for s in range(GROUP):
                    j = g * GROUP + s
                    nc.tensor.matmul(
                        out=d2[:, s * CHUNK : (s + 1) * CHUNK],
                        lhsT=q_sb[:, j * TILE : (j + 1) * TILE],
                        rhs=c_sb[:, s * CHUNK : (s + 1) * CHUNK],
                        start=True,
                        stop=True,
                    )
                nc.vector.tensor_reduce(
                    out=pm_sb[:, g * GROUP : (g + 1) * GROUP],
                    in_=d2.rearrange("p (s c) -> p s c", c=CHUNK),
                    axis=mybir.AxisListType.X,
                    op=mybir.AluOpType.min,
                )
            nc.sync.dma_start(out=pm[:, :], in_=pm_sb)
    _PROGRAMS[n_groups] = nc
    return nc


def _split3(a):
    a = np.asarray(a, np.float32)
    h = a.astype(bf16)
    r1 = (a - h.astype(np.float32)).astype(np.float32)
    m = r1.astype(bf16)
    l = (r1 - m.astype(np.float32)).astype(bf16)
    return h, m, l


def _stack_split(stat4, mov4):
    # product groups hh, hm, mh: error ~2^-18 of term magnitudes (~4e-6
    # absolute here) - far inside the harness tolerance.
    sh, sm, _ = _split3(stat4)
    mh, mm_, _ = _split3(mov4)
    stat = np.concatenate([sh, sh, sm], axis=0).astype(bf16)
    mov = np.concatenate([mh, mm_, mh], axis=0).astype(bf16)
    return stat, mov


def _build_db(ds):
    n = len(ds)
    o1 = np.argsort(ds[:, 1], kind="stable")
    s = ds[o1]
    starts = (np.arange(R_ROWS + 1) * n) // R_ROWS
    out = np.empty_like(s)
    for r in range(R_ROWS):
        seg = s[starts[r] : starts[r + 1]]
        out[starts[r] : starts[r + 1]] = seg[np.argsort(seg[:, 0], kind="stable")]
    edges = np.empty(R_ROWS + 1, np.float64)
    edges[0] = -np.inf
    for r in range(1, R_ROWS):
        edges[r] = 0.5 * (float(s[starts[r] - 1, 1]) + float(s[starts[r], 1]))
    edges[R_ROWS] = np.inf
    return out, starts, edges


def _plan_direction(qs_raw, ds_raw):
    """Returns dict with sorted queries, candidate chunk indices per tile,
    and the coverage metadata for the conservative check."""
    db, starts, edges = _build_db(ds_raw)
    d0lo, d0hi = float(db[:, 0].min()), float(db[:, 0].max())
    d1lo, d1hi = float(db[:, 1].min()), float(db[:, 1].max())
    qc = np.stack(
        [np.clip(qs_raw[:, 0], d0lo, d0hi), np.clip(qs_raw[:, 1], d1lo, d1hi)], -1
    ).astype(np.float32)
    # probe: NN-distance upper bound from a coarse subsample
    S = db[::16]
    qn = (qc * qc).sum(1)
    sn = (S * S).sum(1)
    ub2 = np.maximum((qn[:, None] - 2.0 * (qc @ S.T) + sn[None, :]).min(1), 0)
    ub = np.sqrt(ub2.astype(np.float64))
    dist_out = np.sqrt(((qs_raw - qc) ** 2).sum(1).astype(np.float64))
    wq = np.sqrt(ub * ub + 2.0 * dist_out * ub)  # exact NN ball about clamp(q)
    qrow = np.searchsorted(edges[1:-1], qs_raw[:, 1], "right")
    oq = np.lexsort((qc[:, 0], qrow))
    qs = qs_raw[oq]
    qcs = qc[oq]
    wqs = wq[oq]
    n_t = len(qs) // TILE
    tiles = []
    for t in range(n_t):
        sl = slice(t * TILE, (t + 1) * TILE)
        W = float(wqs[sl].max()) * 1.05 + 0.003
        q0lo, q0hi = float(qcs[sl, 0].min()), float(qcs[sl, 0].max())
        q1lo, q1hi = float(qcs[sl, 1].min()), float(qcs[sl, 1].max())
        rlo = int(np.searchsorted(edges[1:-1], q1lo - W, "right"))
        rhi = int(np.searchsorted(edges[1:-1], q1hi + W, "right"))
        runs = []
        for r in range(rlo, rhi + 1):
            a, b = int(starts[r]), int(starts[r + 1])
            l = a + int(np.searchsorted(db[a:b, 0], q0lo - W, "left"))
            h = a + int(np.searchsorted(db[a:b, 0], q0hi + W, "right"))
            runs.append((r, l, h))
        parts = [np.arange(l, h) for (_, l, h) in runs if h > l]
        idx = np.concatenate(parts) if parts else np.zeros(1, np.int64)
        tiles.append({"idx": idx, "W": W, "rlo": rlo, "rhi": rhi, "runs": runs})
    return {
        "db": db,
        "starts": starts,
        "edges": edges,
        "qs": qs,
        "oq": oq,
        "tiles": tiles,
        "ds_raw": ds_raw,
    }


def _check_direction(plan, dmin):
    db, starts, edges = plan["db"], plan["starts"], plan["edges"]
    qs = plan["qs"]
    bad = np.zeros(len(qs), bool)
    for t, tl in enumerate(plan["tiles"]):
        sl = slice(t * TILE, (t + 1) * TILE)
        q = qs[sl]
        dm = dmin[sl]
        rlo, rhi = tl["rlo"], tl["rhi"]
        gaps = np.full((TILE, 4), np.inf, np.float64)
        if np.isfinite(edges[rlo]):
            gaps[:, 0] = q[:, 1] - edges[rlo]
        if np.isfinite(edges[rhi + 1]):
            gaps[:, 1] = edges[rhi + 1] - q[:, 1]
        L, Rv = -np.inf, np.inf
        for (r, l, h) in tl["runs"]:
            a, b = int(starts[r]), int(starts[r + 1])
            if l > a:
                L = max(L, float(db[l - 1, 0]))
            if h < b:
                Rv = min(Rv, float(db[h, 0]))
        if np.isfinite(L):
            gaps[:, 2] = q[:, 0] - L
        if np.isfinite(Rv):
            gaps[:, 3] = Rv - q[:, 0]
        g = np.clip(gaps, 0, None).min(axis=1)
        bad[sl] = dm.astype(np.float64) > g * g
    return bad


_last_in_maps = None


def kernel(input, mask_samples, norm_scale, norm_shift):
    global _last_in_maps
    x3 = np.asarray(input, dtype=np.float32)
    y = np.asarray(mask_samples, dtype=np.float32)[0]
    sc = np.asarray(norm_scale, dtype=np.float32)
    sh = np.asarray(norm_shift, dtype=np.float32)

    cam = (x3 * sc + sh).astype(np.float32)
    pred = (
        np.stack([cam[:, 0] * FX, cam[:, 1] * FY], axis=-1) / cam[:, 2:3]
    ).astype(np.float32)

    plans = [_plan_direction(pred, y), _plan_direction(y, pred)]

    # flat chunk stream over both directions
    chunks = []  # (direction, tile, candidate index array of len CHUNK)
    for di, plan in enumerate(plans):
        for t, tl in enumerate(plan["tiles"]):
            idx = tl["idx"]
            n_ch = max(1, (len(idx) + CHUNK - 1) // CHUNK)
            padded = np.empty(n_ch * CHUNK, np.int64)
            padded[: len(idx)] = idx
            if len(idx) < len(padded):
                padded[len(idx) :] = idx[0]
            for j in range(n_ch):
                chunks.append((di, t, padded[j * CHUNK : (j + 1) * CHUNK]))

    per_core = -(-len(chunks) // (N_CORES * GROUP)) * GROUP  # ceil to GROUP
    n_groups = per_core // GROUP
    total = per_core * N_CORES
    while len(chunks) < total:
        chunks.append(chunks[-1])

    # device input stacks per direction: stationary (query) / moving (cands)
    qstacks, cstacks = [], []
    for di, plan in enumerate(plans):
        qs, db = plan["qs"], plan["db"]
        qn = (qs * qs).sum(1, dtype=np.float32)
        dn = (db * db).sum(1, dtype=np.float32)
        ones_q = np.ones(len(qs), np.float32)
        ones_d = np.ones(len(db), np.float32)
        a4 = np.stack([qs[:, 0], qs[:, 1], qn, ones_q], axis=0)
        b4 = np.stack([-2.0 * db[:, 0], -2.0 * db[:, 1], ones_d, dn], axis=0)
        qa, cb = _stack_split(a4, b4)
        qstacks.append(qa)
        cstacks.append(cb)

    in_maps = []
    for c in range(N_CORES):
        sl = chunks[c * per_core : (c + 1) * per_core]
        qcols = np.concatenate(
            [qstacks[di][:, t * TILE : (t + 1) * TILE] for (di, t, _) in sl], axis=1
        )
        ccols = np.concatenate([cstacks[di][:, ci] for (di, _, ci) in sl], axis=1)
        in_maps.append(
            {
                "qflat": np.ascontiguousarray(qcols),
                "cflat": np.ascontiguousarray(ccols),
            }
        )
    _last_in_maps = in_maps

    nc = _get_program(n_groups)
    res = run_bass_kernel_spmd(nc, in_maps, core_ids=list(range(N_CORES)))

    # combine partial minima per (direction, tile)
    dmins = [np.full(M, np.inf, np.float32), np.full(N, np.inf, np.float32)]
    for j, (di, t, _) in enumerate(chunks[: len(chunks)]):
        c, lj = divmod(j, per_core)
        col = res.results[c]["pm"][:, lj]
        sl = slice(t * TILE, (t + 1) * TILE)
        np.minimum(dmins[di][sl], col, out=dmins[di][sl])

    # conservative coverage check + exact host fixup
    for di, plan in enumerate(plans):
        bad = _check_direction(plan, dmins[di])
        if bad.any():
            qb = plan["qs"][bad]
            ds_raw = plan["ds_raw"]
            dn_all = (ds_raw * ds_raw).sum(1, dtype=np.float32)
            qn_b = (qb * qb).sum(1, dtype=np.float32)
            d2 = (
                qn_b[:, None] - 2.0 * (qb @ ds_raw.T) + dn_all[None, :]
            ).astype(np.float32)
            dmins[di][bad] = d2.min(1)

    loss = np.float32(
        dmins[0].mean(dtype=np.float64) + dmins[1].mean(dtype=np.float64)
    )
    return np.asarray(loss, dtype=np.float32)


if __name__ == "__main__":
    d = np.load("/root/problem/inputs.npz")
    out = kernel(**{k: d[k] for k in d.files})
    print("loss:", out)
